# revision 1
# baseline (speedup 1.0000x reference)
"""Trainium2 Bass kernel for nn_DirPCAPassConv (bidirectional SAGE mean conv).

rst = relu([feat | mean_in(feat) | mean_out(feat)] @ W.T)

Strategy (8 NeuronCores, SPMD single program, graph/data parallel):
  host: balance nodes into 32-column bins (per-bin in/out degree <= 256),
        shard bins across cores; per (quarter, direction) build deduplicated
        bf16 halo tables so gather indices fit int16; emit per-edge slot
        streams (2 tiles of 128 edge slots per bin).
  device: SWDGE dma_gather (1024 idx/instr, 4 queues) pulls edge-source rows
        to SBUF; TensorE merges each 128-edge tile into feature-major PSUM
        mean-aggregates via matmul with a DVE-built (dsl==iota)*inv_deg
        selection matrix; final 3-chunk matmul with W^T k-chunks + fused
        ReLU eviction writes row-major f32 output.
"""

import sys
sys.path.insert(0, "/opt/trn_rl_repo")

import numpy as np
import ml_dtypes

import concourse.bass as bass
import concourse.mybir as mybir
import concourse.tile as tile
from concourse import bacc




BF16 = ml_dtypes.bfloat16

P = 128            # partitions / tile size in edges
BINW = 32          # columns per bin
TPB = 2            # tiles per bin
BIN_CAP = TPB * P  # edge capacity per bin per direction (256)


def balance_bins(in_deg, out_deg, n_bins, rng):
    """Assign nodes (plus virtual pad slots) to bins of exactly BINW nodes such
    that each bin's in-degree and out-degree sums are <= BIN_CAP.
    Returns [n_bins, BINW] array of node ids (-1 for pad slots), or None."""
    n = in_deg.shape[0]
    n_slots = n_bins * BINW
    assert n_slots >= n
    order = np.argsort(-(in_deg + out_deg), kind="stable")
    bin_in = np.zeros(n_bins, dtype=np.int64)
    bin_out = np.zeros(n_bins, dtype=np.int64)
    bin_cnt = np.zeros(n_bins, dtype=np.int64)
    bins = -np.ones((n_bins, BINW), dtype=np.int64)
    # greedy: place heaviest first into feasible bin with min load
    for v in order:
        iv, ov = in_deg[v], out_deg[v]
        feas = (bin_cnt < BINW) & (bin_in + iv <= BIN_CAP) & (bin_out + ov <= BIN_CAP)
        # also must leave room: remaining capacity check is implicit via greedy
        if not feas.any():
            return None
        load = np.where(feas, np.maximum(bin_in + iv, bin_out + ov), np.iinfo(np.int64).max)
        b = int(np.argmin(load))
        bins[b, bin_cnt[b]] = v
        bin_in[b] += iv
        bin_out[b] += ov
        bin_cnt[b] += 1
    return bins


def prep(feat, W, edge_index, n_cores=8, n_quarters=4):
    N, D = feat.shape
    assert D == 128
    src = np.asarray(edge_index[0], dtype=np.int64)
    dst = np.asarray(edge_index[1], dtype=np.int64)
    E = src.shape[0]

    in_deg = np.bincount(dst, minlength=N)
    out_deg = np.bincount(src, minlength=N)

    rng = np.random.default_rng(0)
    # bins_per_core must be divisible by 16 (psum chunks of 16 bins). Quarter
    # boundaries are slot ranges (tile-aligned automatically since slots per
    # quarter = bins_per_core*BIN_CAP/n_quarters and BIN_CAP=256).
    nodes_per_core = (N + n_cores - 1) // n_cores
    bins_per_core = ((nodes_per_core + BINW - 1) // BINW + 15) // 16 * 16
    assert (bins_per_core * BIN_CAP) % (n_quarters * P) == 0
    while True:
        n_bins = bins_per_core * n_cores
        bins = balance_bins(in_deg, out_deg, n_bins, rng)
        if bins is not None:
            break
        bins_per_core += 16

    # deal bins to cores round-robin to balance per-core edge counts
    bin_core = np.arange(n_bins) % n_cores
    # per-core bin lists
    core_bins = [np.where(bin_core == c)[0] for c in range(n_cores)]

    n_cols = bins_per_core * BINW          # columns per core
    n_chunks = bins_per_core // 16         # psum chunks per core
    n_tiles = bins_per_core * TPB          # tiles per core per dir
    bins_per_q = bins_per_core // n_quarters

    # edge lists per owner node, per direction
    # dir 0 (fwd): owner=dst, gathered=src ; dir 1 (bwd): owner=src, gathered=dst
    def owner_sorted(owner, other):
        o = np.argsort(owner, kind="stable")
        return owner[o], other[o]

    f_own, f_oth = owner_sorted(dst, src)
    b_own, b_oth = owner_sorted(src, dst)
    f_start = np.searchsorted(f_own, np.arange(N + 1))
    b_start = np.searchsorted(b_own, np.arange(N + 1))

    cores = []
    TQ_max = 0
    for c in range(n_cores):
        cb = core_bins[c]                  # bin ids (global) in core order
        node_of_col = -np.ones(n_cols, dtype=np.int64)
        for j, b in enumerate(cb):
            node_of_col[j * BINW:(j + 1) * BINW] = bins[b]

        dirs = []
        for d, (e_start, e_oth, degs) in enumerate(
            ((f_start, f_oth, in_deg), (b_start, b_oth, out_deg))
        ):
            # per-slot arrays
            n_slots = n_tiles * P
            slot_src = np.zeros(n_slots, dtype=np.int64)   # gathered global node id
            slot_dsl = -np.ones(n_slots, dtype=np.float32) # col within bin, -1 pad
            slot_einv = np.zeros(n_slots, dtype=np.float32)
            for j in range(bins_per_core):
                base = j * BIN_CAP
                fill = 0
                for w in range(BINW):
                    v = node_of_col[j * BINW + w]
                    if v < 0:
                        continue
                    s0, s1 = e_start[v], e_start[v + 1]
                    k = s1 - s0
                    if k == 0:
                        continue
                    inv = 1.0 / max(degs[v], 1)
                    slot_src[base + fill: base + fill + k] = e_oth[s0:s1]
                    slot_dsl[base + fill: base + fill + k] = w
                    slot_einv[base + fill: base + fill + k] = inv
                    fill += k
                assert fill <= BIN_CAP
            # quarter tables: dedup per quarter
            q_tables = []
            slot_idx = np.zeros(n_slots, dtype=np.int64)
            slots_per_q = bins_per_q * BIN_CAP
            for q in range(n_quarters):
                sl = slice(q * slots_per_q, (q + 1) * slots_per_q)
                uniq, inv_map = np.unique(slot_src[sl], return_inverse=True)
                slot_idx[sl] = inv_map
                q_tables.append(uniq)
                TQ_max = max(TQ_max, uniq.shape[0])
            dirs.append(dict(slot_src=slot_src, slot_dsl=slot_dsl,
                             slot_einv=slot_einv, slot_idx=slot_idx,
                             q_tables=q_tables))
        cores.append(dict(node_of_col=node_of_col, dirs=dirs))

    meta = dict(n_cores=n_cores, n_quarters=n_quarters, n_cols=n_cols,
                n_chunks=n_chunks, n_tiles=n_tiles, bins_per_core=bins_per_core,
                bins_per_q=bins_per_q, TQ=TQ_max, N=N, D=D)
    return cores, meta


def build_core_inputs(feat, W, cores, meta, c):
    """Build the input arrays for core c (all dtypes device-ready)."""
    N, D = meta["N"], meta["D"]
    TQ = meta["TQ"]
    nq = meta["n_quarters"]
    n_tiles = meta["n_tiles"]
    n_cols = meta["n_cols"]
    n_slots = n_tiles * P

    feat_bf = feat.astype(BF16)
    co = cores[c]

    tables = np.zeros((2, nq, TQ, D), dtype=BF16)
    idx = np.zeros((2, 128, n_slots // 16), dtype=np.int16)
    dsl = np.zeros((2, P, n_tiles), dtype=BF16)
    einv = np.zeros((2, P, n_tiles), dtype=BF16)
    for d in range(2):
        dd = co["dirs"][d]
        for q in range(nq):
            t = dd["q_tables"][q]
            tables[d, q, :t.shape[0]] = feat_bf[t]
        # idx wrapped: idx for slot i at [i%16, i//16]; 16-row block
        # replicated across all 128 partitions (8 Q7 cores x 16)
        si = dd["slot_idx"].astype(np.int16)
        idx[d] = np.tile(si.reshape(-1, 16).T, (8, 1))
        dsl[d] = dd["slot_dsl"].reshape(n_tiles, P).T.astype(BF16)
        einv[d] = dd["slot_einv"].reshape(n_tiles, P).T.astype(BF16)

    noc = co["node_of_col"]
    featT = np.zeros((D, n_cols), dtype=BF16)
    real = noc >= 0
    featT[:, real] = feat_bf[noc[real]].T

    # W [128 out, 384 in]; k-chunks of W^T: wkt[k, kc, o] = W[o, kc*128+k]
    wkt = np.ascontiguousarray(
        W.reshape(128, 3, 128).transpose(2, 1, 0)).astype(BF16)

    iota = np.broadcast_to(
        np.tile(np.arange(BINW, dtype=np.float32), 8), (P, 8 * BINW)
    ).astype(BF16).copy()

    return dict(tables=tables, idx=idx, dsl=dsl, einv=einv,
                featT=featT, wkt=wkt, iota=iota)


def numpy_simulate_core(inp, meta):
    """Simulate exactly what the device kernel computes for one core, in numpy."""
    n_tiles = meta["n_tiles"]
    n_chunks = meta["n_chunks"]
    nq = meta["n_quarters"]
    n_cols = meta["n_cols"]
    TQ = meta["TQ"]
    slots_per_q = n_tiles * P // nq

    hT = np.zeros((2, 128, n_cols), dtype=np.float32)
    for d in range(2):
        # gather
        idx_unwrapped = inp["idx"][d][:16].T.reshape(-1).astype(np.int64)  # [n_slots]
        X = np.zeros((n_tiles * P, 128), dtype=np.float32)
        for q in range(nq):
            sl = slice(q * slots_per_q, (q + 1) * slots_per_q)
            X[sl] = inp["tables"][d, q][idx_unwrapped[sl]].astype(np.float32)
        for g in range(n_tiles):
            xt = X[g * P:(g + 1) * P]                       # [128 edges, 128 feat]
            dslv = inp["dsl"][d][:, g].astype(np.float32)   # [128]
            einvv = inp["einv"][d][:, g].astype(np.float32)
            M = (dslv[:, None] == np.arange(BINW)[None, :]).astype(np.float32)
            M = (M.astype(BF16).astype(np.float32) *
                 einvv[:, None]).astype(BF16).astype(np.float32)
            bin_id = g // TPB
            c0 = bin_id * BINW
            hT[d][:, c0:c0 + BINW] += xt.T @ M
    hT = hT.astype(BF16).astype(np.float32)

    featT = inp["featT"].astype(np.float32)
    wkt = inp["wkt"].astype(np.float32)  # [128k, 3, 128o]
    out = np.zeros((n_cols, 128), dtype=np.float32)
    for nt in range(n_cols // 128):
        sl = slice(nt * 128, (nt + 1) * 128)
        acc = featT[:, sl].T @ wkt[:, 0, :]
        acc = acc + hT[0][:, sl].T @ wkt[:, 1, :]
        acc = acc + hT[1][:, sl].T @ wkt[:, 2, :]
        out[sl] = np.maximum(acc, 0.0)
    return out





GRP = 8  # tiles per M-build DVE group


def build_nc(meta, t_slab=8, n_queues=4, slab_bufs=8, enable_asserts=False):
    nq = meta["n_quarters"]
    TQ = meta["TQ"]
    n_tiles = meta["n_tiles"]
    n_cols = meta["n_cols"]
    n_chunks = meta["n_chunks"]
    n_slots = n_tiles * P
    tiles_per_q = n_tiles // nq
    if tiles_per_q % t_slab != 0:
        # pick the largest divisor of tiles_per_q that is <= t_slab
        t_slab = max(d for d in range(1, t_slab + 1) if tiles_per_q % d == 0)
    assert n_tiles % GRP == 0
    bf16 = mybir.dt.bfloat16
    f32 = mybir.dt.float32

    nc = bacc.Bacc(
        "TRN2", target_bir_lowering=False, debug=False,
        enable_asserts=enable_asserts, num_swdge_queues=n_queues,
    )
    tables = nc.dram_tensor("tables", [2, nq, TQ, P], bf16, kind="ExternalInput")
    idx = nc.dram_tensor("idx", [2, P, n_slots // 16], mybir.dt.int16,
                         kind="ExternalInput")
    dsl = nc.dram_tensor("dsl", [2, P, n_tiles], bf16, kind="ExternalInput")
    einv = nc.dram_tensor("einv", [2, P, n_tiles], bf16, kind="ExternalInput")
    featT = nc.dram_tensor("featT", [P, n_cols], bf16, kind="ExternalInput")
    wkt = nc.dram_tensor("wkt", [P, 3, P], bf16, kind="ExternalInput")
    iota = nc.dram_tensor("iota", [P, GRP * BINW], bf16, kind="ExternalInput")
    out = nc.dram_tensor("out", [n_cols, P], f32, kind="ExternalOutput")

    with tile.TileContext(nc) as tc:
        with (
            tc.tile_pool(name="const", bufs=1) as constp,
            tc.tile_pool(name="slab", bufs=slab_bufs) as slabp,
            tc.tile_pool(name="m", bufs=4) as mp,
            tc.tile_pool(name="agg", bufs=3) as aggp,
            tc.tile_pool(name="ost", bufs=2) as ostp,
            tc.tile_pool(name="pm", bufs=4, space="PSUM") as pmp,
            tc.tile_pool(name="pf", bufs=2, space="PSUM") as pfp,
        ):
            featT_sb = constp.tile([P, n_cols], bf16)
            nc.sync.dma_start(featT_sb[:], featT[:])
            wkt_sb = constp.tile([P, 3, P], bf16)
            nc.sync.dma_start(wkt_sb[:], wkt[:])
            iota_sb = constp.tile([P, GRP * BINW], bf16)
            nc.sync.dma_start(iota_sb[:], iota[:])
            idx_sb, dsl_sb, einv_sb = [], [], []
            for d in range(2):
                t1 = constp.tile([P, n_slots // 16], mybir.dt.int16, tag=f"idx{d}")
                nc.sync.dma_start(t1[:], idx[d])
                idx_sb.append(t1)
                t2 = constp.tile([P, n_tiles], bf16, tag=f"dsl{d}")
                nc.sync.dma_start(t2[:], dsl[d])
                dsl_sb.append(t2)
                t3 = constp.tile([P, n_tiles], bf16, tag=f"einv{d}")
                nc.sync.dma_start(t3[:], einv[d])
                einv_sb.append(t3)

            slab_cur = [None, None]
            m_cur = [None, None]
            for ci in range(n_chunks):
                hts = []
                for d in range(2):
                    ps = pmp.tile([P, 16 * BINW], f32, tag="pm")
                    for b16 in range(16):
                        for t in range(TPB):
                            g = (ci * 16 + b16) * TPB + t
                            if g % t_slab == 0:
                                s = g // t_slab
                                slab_cur[d] = slabp.tile(
                                    [P, t_slab, P], bf16, tag=f"slab{d}",
                                    name=f"slab{d}_{g}")
                                q = (s * t_slab) // tiles_per_q
                                nidx = t_slab * P
                                i0 = s * (nidx // 16)
                                nc.gpsimd.dma_gather(
                                    slab_cur[d][:], tables[d, q],
                                    idx_sb[d][:, i0:i0 + nidx // 16],
                                    nidx, nidx, P,
                                    queue_num=(2 * s + d) % n_queues)
                            if g % GRP == 0:
                                m01 = mp.tile([P, GRP, BINW], bf16, tag="m01")
                                msc = mp.tile([P, GRP, BINW], bf16, tag="msc")
                                nc.vector.tensor_tensor(
                                    out=m01[:],
                                    in0=dsl_sb[d][:, g:g + GRP, None]
                                        .to_broadcast([P, GRP, BINW]),
                                    in1=iota_sb[:].rearrange(
                                        "p (t w) -> p t w", w=BINW),
                                    op=mybir.AluOpType.is_equal)
                                nc.vector.tensor_tensor(
                                    out=msc[:], in0=m01[:],
                                    in1=einv_sb[d][:, g:g + GRP, None]
                                        .to_broadcast([P, GRP, BINW]),
                                    op=mybir.AluOpType.mult)
                                m_cur[d] = msc
                            nc.tensor.matmul(
                                out=ps[:, b16 * BINW:(b16 + 1) * BINW],
                                lhsT=slab_cur[d][:, g % t_slab, :],
                                rhs=m_cur[d][:, g % GRP, :],
                                start=(b16 == 0 and t == 0),
                                stop=(b16 == 15 and t == TPB - 1))
                    ht = aggp.tile([P, 16 * BINW], bf16, tag="ht")
                    nc.vector.tensor_copy(out=ht[:], in_=ps[:])
                    hts.append(ht)
                ost = ostp.tile([P, 4, P], f32)
                for nt in range(4):
                    pf = pfp.tile([P, P], f32, tag="pf")
                    c0 = ci * 512 + nt * P
                    nc.tensor.matmul(out=pf[:], lhsT=featT_sb[:, c0:c0 + P],
                                     rhs=wkt_sb[:, 0, :], start=True, stop=False)
                    nc.tensor.matmul(out=pf[:], lhsT=hts[0][:, nt * P:(nt + 1) * P],
                                     rhs=wkt_sb[:, 1, :], start=False, stop=False)
                    nc.tensor.matmul(out=pf[:], lhsT=hts[1][:, nt * P:(nt + 1) * P],
                                     rhs=wkt_sb[:, 2, :], start=False, stop=True)
                    nc.scalar.activation(ost[:, nt, :], pf[:],
                                         mybir.ActivationFunctionType.Relu)
                nc.sync.dma_start(
                    out[ci * 512:(ci + 1) * 512, :]
                        .rearrange("(t p) o -> p t o", p=P),
                    ost[:])
    nc.compile()
    return nc


_nc_cache = {}


def kernel(feat, W, edge_index, trace=False, tmpdir=None, trace_cores=None):
    """Full-input bidirectional-SAGE forward on 8 NeuronCores.

    Nodes are sharded across cores via a load-balanced permutation; per-core
    inputs carry per-quarter deduplicated bf16 halo tables, edge-slot
    metadata, and feature-major bf16 features. The output is reassembled to
    the original node order on the host.
    """
    feat = np.asarray(feat, dtype=np.float32)
    W = np.asarray(W, dtype=np.float32)
    edge_index = np.asarray(edge_index)
    n_cores = 8

    cores, meta = prep(feat, W, edge_index, n_cores=n_cores)
    key = (meta["n_cols"], meta["n_tiles"], meta["TQ"])
    if key not in _nc_cache:
        _nc_cache[key] = build_nc(meta)
    nc = _nc_cache[key]

    in_maps = [build_core_inputs(feat, W, cores, meta, c) for c in range(n_cores)]

    from concourse.bass_utils import run_bass_kernel_spmd
    res = run_bass_kernel_spmd(
        nc, in_maps, core_ids=list(range(n_cores)),
        trace=trace, tmpdir=tmpdir, trace_cores=trace_cores,
    )

    out_full = np.zeros((meta["N"], 128), dtype=np.float32)
    for c in range(n_cores):
        noc = cores[c]["node_of_col"]
        real = noc >= 0
        out_full[noc[real]] = res.results[c]["out"][real]
    kernel.last_results = res
    kernel.last_meta = meta
    return out_full

